# revision 43
# baseline (speedup 1.0000x reference)
"""Trainium2 Bass kernel for nn_MultiHeadAttention (B=4, S=2048, D=1024,
H=16, DK=DV=64) with key-padding + causal mask, exp-without-max softmax.

Sharding: 8 cores = (batch b = core//2) x (head half = core%2, 8 heads each).
Each core computes its batch's projections for its 8 heads and the full
attention for those heads; host reassembles [B, S, H*DV].

Design (per core), all matmul operands bf16 (PSUM accumulates fp32):
 - KEY PACKING: the key-padding mask multiplies scores AFTER exp, so masked
   keys contribute nothing anywhere. The host gathers each batch's valid
   keys into a contiguous prefix (zero-padded to a fixed SP), so K/V
   projections and the whole attention run on ~51% of the keys. Causality
   against ORIGINAL key positions becomes per-batch 0/1 boundary tiles
   (host-precomputed) multiplied into E only where a tile straddles the
   causal boundary; fully-causal tiles need no mask at all. The k-tile trip
   counts / masked set / column trims are derived from the mask and baked
   into the program (cache-keyed).
 - host passes X^T = Q[b].T etc. [D, S] so the contraction dim (D) lands on
   partitions; W^T likewise. qT/kT computed as W^T-chunk.T @ X^T-chunk ->
   [feat, S]; v computed natural [S, feat].
 - mv1 = [v | 1] per k-tile: the ones column rides the PV matmul (M=65) and
   yields the masked denominator row for free (padded/masked rows have
   E == 0, so plain ones are exact).
 - scores^T tile [128 k, <=512 q] = kT-tile.T @ qT-chunk; ACT exp with
   scale=1/8 fused; boundary masks on DVE.
 - q^T per head zero-padded to K=128 partitions (the co-packed other
   head's kT rows are cancelled by the zeros; keeps the PE at full rate).
 - normalization without PE transposes: ctx^T stays feature-major. The
   sums row is DMA-regathered to [4,128], reciprocal_approx_fast'd,
   scattered into row 0 of a zeroed [128,512] tile, and broadcast to all
   partitions with one K=128 all-ones fp32r matmul; a DVE multiply then
   scales ctx^T and the output DMAs out feature-major. The host transposes
   the per-core [FPC, S] output during unshard.
 - software pipeline: project 512 tokens, then attend at that q-chunk for
   all heads, with the next group's projections emitted between head pairs
   so the Tile scheduler fills ACT-paced gaps with projection matmuls.
"""

import sys

sys.path.insert(0, "/opt/trn_rl_repo")

import ml_dtypes
import numpy as np

import concourse.bass as bass
import concourse.mybir as mybir
import concourse.tile as tile
from concourse import bacc
from concourse.bass_utils import run_bass_kernel_spmd

F32 = mybir.dt.float32
F32R = mybir.dt.float32r
BF16 = mybir.dt.bfloat16
EXP = mybir.ActivationFunctionType.Exp
COPY = mybir.ActivationFunctionType.Copy
IDENT_FN = mybir.ActivationFunctionType.Identity

B, S, D = 4, 2048, 1024
H, DK, DV = 16, 64, 64
HPC = 8  # heads per core
FPC = HPC * DK  # projected features per core (512)
NQC = S // 512  # 4 q-chunks
TC = 512  # projection token-chunk size
NDC = D // 128  # 8 contraction chunks
SCALE = 1.0 / np.sqrt(DK)
BF = np.dtype(ml_dtypes.bfloat16)


def build_nc(flags, SP, NKT, masked_offs):
    """flags: (has_bq, has_bk, has_bv). SP: padded packed-key count.
    NKT[j]: k-tile trip count for q-chunk j. masked_offs: ordered tuple of
    ((j, kt), off) for tiles needing a boundary mask, `off` = first valid
    q column (matmul/exp trimmed to [off:512])."""
    has_bq, has_bk, has_bv = flags
    NTP = SP // 128  # packed k tiles
    midx = {jk: i for i, (jk, _) in enumerate(masked_offs)}
    offs = dict(masked_offs)
    NM = len(masked_offs)
    # kv projection chunk widths
    KV_CW = [min(TC, SP - c * TC) for c in range((SP + TC - 1) // TC)]

    nc = bacc.Bacc()

    xt = {"q": nc.dram_tensor("xtq", [D, S], BF16, kind="ExternalInput")}
    for n in "kv":
        xt[n] = nc.dram_tensor(f"xt{n}", [D, SP], BF16, kind="ExternalInput")
    wt = {n: nc.dram_tensor(f"wt{n}", [D, FPC], BF16, kind="ExternalInput") for n in "qkv"}
    trim_d = nc.dram_tensor("trim", [max(NM, 1), 128, 512], BF16, kind="ExternalInput")
    ones_d = nc.dram_tensor("ones", [128], F32R, kind="ExternalInput")
    onesb_d = nc.dram_tensor("onesb", [128], BF16, kind="ExternalInput")
    zeros_d = nc.dram_tensor("zeros", [512], F32R, kind="ExternalInput")
    bq_d = nc.dram_tensor("bq", [FPC], F32, kind="ExternalInput") if has_bq else None
    bk_d = nc.dram_tensor("bk", [FPC], F32, kind="ExternalInput") if has_bk else None
    bv_d = nc.dram_tensor("bv", [DV], F32, kind="ExternalInput") if has_bv else None
    out_d = nc.dram_tensor("out", [FPC, S], F32R, kind="ExternalOutput")

    with tile.TileContext(nc) as tc:
        with (
            tc.tile_pool(name="const", bufs=1) as cpool,
            tc.tile_pool(name="wtp", bufs=1) as wtpool,
            tc.tile_pool(name="xtp", bufs=6) as xtpool,
            tc.tile_pool(name="big", bufs=1) as big,
            tc.tile_pool(name="qt", bufs=2) as qtpool,
            tc.tile_pool(name="e", bufs=4) as epool,
            tc.tile_pool(name="fin", bufs=4) as fin,
            tc.tile_pool(name="mm", bufs=6, space="PSUM") as psmm,
            tc.tile_pool(name="ctx", bufs=2, space="PSUM") as psctx,
        ):
            # ---------------- constants
            trim = cpool.tile([128, max(NM, 1), 512], BF16)
            # full 128-col all-ones weight (128 cols -> FWL-eligible LDW)
            ones128 = cpool.tile([128, 128], F32R)
            nc.sync.dma_start(
                out=ones128,
                in_=bass.AP(tensor=ones_d, offset=0, ap=[[0, 128], [1, 128]]),
            )
            ones_col = cpool.tile([128, 1], BF16)
            nc.sync.dma_start(
                out=ones_col,
                in_=bass.AP(tensor=onesb_d, offset=0, ap=[[1, 128], [128, 1]]),
            )
            # two alternating recip-scatter tiles (one per head of a pair) so
            # the deferred broadcast of pair p doesn't serialize pair p+1's
            # recip chain
            rec1z = []
            for i in range(2):
                t = cpool.tile([128, 512], F32R, name=f"rec1z{i}")
                nc.sync.dma_start(
                    out=t,
                    in_=bass.AP(tensor=zeros_d, offset=0, ap=[[0, 128], [1, 512]]),
                )
                rec1z.append(t)
            zeros_sb = cpool.tile([128, 512], F32)
            nc.vector.memset(zeros_sb, 0.0)
            bias_sb = {}
            for n, b_d in (("q", bq_d), ("k", bk_d)):
                if b_d is not None:
                    t = cpool.tile([128, 4], F32)
                    nc.sync.dma_start(
                        out=t, in_=bass.AP(tensor=b_d, offset=0, ap=[[1, 128], [128, 4]])
                    )
                    bias_sb[n] = t
            if bv_d is not None:
                bv_b = cpool.tile([128, FPC], F32)
                nc.sync.dma_start(
                    out=bv_b,
                    in_=bass.AP(tensor=bv_d, offset=0, ap=[[0, 128], [0, HPC], [1, DV]]),
                )

            # PE warm-up: dummy matmuls on the early-arriving (tiny) constant
            # tiles hold the PE activity monitor at full clock through the
            # initial input-DMA wait. Results never read; not DCE'd.
            for wi in range(24):
                wps = psmm.tile([128, 512], F32, tag="mm", name=f"warm{wi}")
                nc.tensor.matmul(
                    wps, lhsT=rec1z[0][:, 0:128], rhs=rec1z[0], start=True, stop=True
                )

            # persistent projection outputs (kT/mv1 span all packed tokens;
            # qT only lives for its own 512-token q-chunk group).
            # mv1 holds per-head [v | 1] at 65-col stride plus a 63-col zero
            # tail: the PV lhsT reads a 128-col window (spilling into the
            # next head's v; the resulting ctx rows 65..127 are never read)
            # so LDWEIGHTS is FWL-eligible and hides under the matmul stream.
            kT_all = big.tile([128, 4, SP], BF16)
            mv1 = big.tile([128, NTP, HPC * (DV + 1) + 63], BF16)
            for t in range(NTP):
                nc.vector.tensor_copy(
                    mv1[:, t, HPC * (DV + 1) :], zeros_sb[:, 0:63]
                )

            # weights loaded lazily at first use (interleaved per-dc with the
            # first x chunk so the first matmul's operands arrive first)
            w_sb = {}

            def dma_chunk(name, tcn, cw, sdim):
                """Allocate x tiles for one token chunk and issue its DMAs
                (plus the weight DMAs, dc-interleaved, on first use)."""
                first_w = name not in w_sb
                if first_w:
                    w_sb[name] = wtpool.tile(
                        [128, NDC, FPC], BF16, tag=f"w{name}", name=f"w{name}"
                    )
                halves = [
                    xtpool.tile([128, 4, TC], BF16, tag="x", name=f"x{name}{tcn}l"),
                    xtpool.tile([128, 4, TC], BF16, tag="x", name=f"x{name}{tcn}h"),
                ]
                x_at = lambda dc: halves[dc // 4][:, dc % 4, :cw]
                if first_w:
                    for dc in range(NDC):
                        nc.sync.dma_start(
                            out=w_sb[name][:, dc, :],
                            in_=bass.AP(
                                tensor=wt[name],
                                offset=dc * 128 * FPC,
                                ap=[[FPC, 128], [1, FPC]],
                            ),
                        )
                        nc.sync.dma_start(
                            out=x_at(dc),
                            in_=bass.AP(
                                tensor=xt[name],
                                offset=tcn * TC + dc * 128 * sdim,
                                ap=[[sdim, 128], [1, cw]],
                            ),
                        )
                else:
                    for hv in range(2):
                        nc.sync.dma_start(
                            out=halves[hv][:, :, :cw],
                            in_=bass.AP(
                                tensor=xt[name],
                                offset=tcn * TC + hv * 4 * 128 * sdim,
                                ap=[[sdim, 128], [128 * sdim, 4], [1, cw]],
                            ),
                        )
                return x_at

            def q_fc_group(x_at, fc, qT_j):
                ps = psmm.tile([128, TC], F32, tag="mm")
                for dc in range(NDC):
                    nc.tensor.matmul(
                        ps,
                        lhsT=w_sb["q"][:, dc, fc * 128 : (fc + 1) * 128],
                        rhs=x_at(dc),
                        start=(dc == 0),
                        stop=(dc == NDC - 1),
                    )
                # split into per-head zero-padded blocks (DVE keeps ACT
                # free for exp)
                for hh in range(2):
                    hp = hh * 64
                    o = qT_j[hp : hp + 64, 2 * fc + hh, :]
                    i_ = ps[hp : hp + 64, :]
                    if "q" in bias_sb:
                        nc.vector.tensor_scalar_add(
                            o, i_, bias_sb["q"][hp : hp + 64, fc : fc + 1]
                        )
                    else:
                        nc.vector.tensor_copy(o, i_)

            def k_fc_group(x_at, tcn, cw, fc):
                ps = psmm.tile([128, TC], F32, tag="mm")
                for dc in range(NDC):
                    nc.tensor.matmul(
                        ps[:, :cw],
                        lhsT=w_sb["k"][:, dc, fc * 128 : (fc + 1) * 128],
                        rhs=x_at(dc),
                        start=(dc == 0),
                        stop=(dc == NDC - 1),
                    )
                o = kT_all[:, fc, tcn * TC : tcn * TC + cw]
                if "k" in bias_sb:
                    nc.scalar.activation(
                        o, ps[:, :cw], IDENT_FN, bias=bias_sb["k"][:, fc : fc + 1]
                    )
                else:
                    nc.scalar.activation(o, ps[:, :cw], COPY)

            def v_tt_group(x_at, tcn, tt):
                t = tcn * (TC // 128) + tt
                ps = psmm.tile([128, FPC], F32, tag="mm")
                for dc in range(NDC):
                    nc.tensor.matmul(
                        ps,
                        lhsT=x_at(dc)[:, tt * 128 : (tt + 1) * 128],
                        rhs=w_sb["v"][:, dc, :],
                        start=(dc == 0),
                        stop=(dc == NDC - 1),
                    )
                if bv_d is not None:
                    nc.vector.tensor_add(ps, ps, bv_b)
                for h in range(HPC):
                    base = h * (DV + 1)
                    # ACT copy: DVE is the busier engine during projections
                    nc.scalar.activation(
                        mv1[:, t, base : base + DV],
                        ps[:, h * DV : (h + 1) * DV],
                        COPY,
                    )
                    nc.gpsimd.tensor_copy(
                        mv1[:, t, base + DV : base + DV + 1], ones_col
                    )

            def project_q_chunk(tcn, qT_j):
                x_at = dma_chunk("q", tcn, TC, S)
                for fc in range(4):
                    q_fc_group(x_at, fc, qT_j)

            def project_kv_chunk(tcn):
                cw = KV_CW[tcn]
                x_at_v = dma_chunk("v", tcn, cw, SP)
                for tt in range(cw // 128):
                    v_tt_group(x_at_v, tcn, tt)
                x_at_k = dma_chunk("k", tcn, cw, SP)
                for fc in range(4):
                    k_fc_group(x_at_k, tcn, cw, fc)

            def finalize_pre(h, j, hh, ctx):
                """Move ctx to SBUF (releases the PSUM bank) and start the
                off-PE recip chain: sums regather -> +eps -> approx recip ->
                scatter into rec1z[hh] row 0."""
                ctx_sb = fin.tile([DV + 1, 512], F32R, tag="ctxsb", bufs=4)
                nc.vector.tensor_copy(ctx_sb, ctx[0 : DV + 1, :])
                sums4 = fin.tile([4, 128], F32R, tag="sums", bufs=2)
                nc.sync.dma_start(
                    out=sums4,
                    in_=ctx_sb[DV : DV + 1, :].rearrange("p (a b) -> p a b", a=4),
                )
                rec = fin.tile([4, 128], F32, tag="rec", bufs=2)
                nc.vector.tensor_scalar_add(rec, sums4, 1e-8)
                rec2 = fin.tile([4, 128], F32, tag="rec2", bufs=2)
                nc.vector.reciprocal_approx_fast(out=rec2, in_=rec)
                # gpsimd DGE: the f32 -> f32r retag counts as a casting DMA
                nc.gpsimd.dma_start(
                    out=rec1z[hh][0:1, :].rearrange("p (a b) -> p a b", a=4),
                    in_=rec2,
                )
                return (h, j, hh, ctx_sb)

            def finalize_post(h, j, hh, ctx_sb):
                """Broadcast recip to all partitions with an all-ones matmul,
                scale ctx^T, and DMA the feature-major block out. Emitted one
                attention-pair late so the recip chain's DMA latency hides
                behind the next pair's matmuls instead of stalling the PE."""
                bps = psmm.tile([128, 512], F32, tag="mm", name=f"bc{h}_{j}")
                nc.tensor.matmul(
                    bps, lhsT=ones128, rhs=rec1z[hh], start=True, stop=True
                )
                out_sb = fin.tile([DV, 512], F32R, tag="outsb", bufs=6)
                nc.vector.tensor_mul(out_sb, ctx_sb[0:DV, :], bps[0:DV, :])
                nc.sync.dma_start(
                    out=out_d[h * DV : (h + 1) * DV, j * 512 : (j + 1) * 512],
                    in_=out_sb,
                )

            def attention_pair(fc, j, qT_j, posts):
                h0, h1 = 2 * fc, 2 * fc + 1
                nkt = NKT[j]
                ctxs = [
                    psctx.tile([128, 512], F32, tag="ctx", name=f"ctx{fc}_{j}_{hh}")
                    for hh in range(2)
                ]
                def emit_pv(kt, off, Es):
                    for hh, h in enumerate((h0, h1)):
                        base = h * (DV + 1)
                        nc.tensor.matmul(
                            ctxs[hh][:, off:],
                            lhsT=mv1[:, kt, base : base + 128],
                            rhs=Es[hh][:, off:],
                            start=(kt == 0),
                            stop=(kt == nkt - 1),
                        )

                # one-step QK lookahead: kt+1's QK is emitted before kt's PV
                # so the PE always has a ready matmul while exp/mask run
                pend = None
                for kt in range(nkt):
                    mi = midx.get((j, kt))
                    off = offs.get((j, kt), 0)
                    qs = slice(off, 512)
                    scs = []
                    for hh, h in enumerate((h0, h1)):
                        sc = psmm.tile([128, 512], F32, tag="mm", name=f"sc{fc}_{j}_{kt}_{hh}")
                        nc.tensor.matmul(
                            sc[:, off:],
                            lhsT=kT_all[:, fc, kt * 128 : (kt + 1) * 128],
                            rhs=qT_j[:, h, qs],
                            start=True,
                            stop=True,
                        )
                        scs.append(sc)
                    Es = []
                    for hh, h in enumerate((h0, h1)):
                        E = epool.tile([128, 512], BF16, tag="e", name=f"e{fc}_{j}_{kt}_{hh}")
                        nc.scalar.activation(E[:, off:], scs[hh][:, off:], EXP, scale=float(SCALE))
                        if mi is not None:
                            # gpsimd: all-SBUF operands, and both DVE and ACT
                            # are busier engines during attention
                            nc.gpsimd.tensor_mul(E[:, off:], E[:, off:], trim[:, mi, off:])
                        Es.append(E)
                    if pend is not None:
                        emit_pv(*pend)
                    pend = (kt, off, Es)
                emit_pv(*pend)
                # previous pair's deferred finalize lands behind this pair's
                # matmuls in the PE queue
                for p in posts:
                    finalize_post(*p)
                return [
                    finalize_pre(h, j, hh, ctxs[hh]) for hh, h in enumerate((h0, h1))
                ]

            # software pipeline: project 512 tokens, then attend at that
            # q-chunk for all heads (next group's projections fill PE gaps).
            # q^T stored per head zero-padded to K=128 partitions: a full-K
            # matmul keeps the PE clock-gate warm and the co-packed other
            # head's kT rows are cancelled by the zeros.
            def new_qtile(j):
                qT_j = qtpool.tile([128, HPC, 512], BF16, tag="qt", name=f"qt{j}")
                for h in range(HPC):
                    zp = 64 - (h % 2) * 64
                    nc.vector.tensor_copy(qT_j[zp : zp + 64, h, :], zeros_sb[0:64, :])
                return qT_j

            NKC = len(KV_CW)
            # kv chunk c is first needed by the q-chunk j where NKT[j] > 4c
            kv_need = {}
            for c in range(NKC):
                js = [j for j in range(NQC) if NKT[j] > 4 * c]
                kv_need[c] = js[0] if js else None

            qtiles = {0: new_qtile(0)}
            project_q_chunk(0, qtiles[0])
            for c in range(NKC):
                if kv_need[c] == 0 or kv_need[c] is None:
                    project_kv_chunk(c)
            # boundary-mask tiles aren't needed until the first attention
            # pair; issuing their (large) DMA after the upfront projection
            # DMAs keeps the pipeline head tight
            nc.sync.dma_start(
                out=trim,
                in_=bass.AP(
                    tensor=trim_d,
                    offset=0,
                    ap=[[512, 128], [512 * 128, max(NM, 1)], [1, 512]],
                ),
            )
            posts = []
            for j in range(NQC):
                qT_j = qtiles[j]
                # stage next-chunk projections as a work list spread evenly
                # across this j's four attention pairs (PE filler during the
                # ACT-paced attention stretches)
                work = []
                if j + 1 < NQC:
                    qtiles[j + 1] = new_qtile(j + 1)
                    x_at_q = dma_chunk("q", j + 1, TC, S)
                    work += [
                        (lambda fc=fc: q_fc_group(x_at_q, fc, qtiles[j + 1]))
                        for fc in range(4)
                    ]
                for c in range(NKC):
                    if kv_need[c] == j + 1:
                        cw = KV_CW[c]
                        x_at_v = dma_chunk("v", c, cw, SP)
                        x_at_k = dma_chunk("k", c, cw, SP)
                        vts = [
                            (lambda tt=tt, x=x_at_v, c=c: v_tt_group(x, c, tt))
                            for tt in range(cw // 128)
                        ]
                        kfs = [
                            (lambda fc=fc, x=x_at_k, c=c, cw=cw: k_fc_group(x, c, cw, fc))
                            for fc in range(4)
                        ]
                        # interleave v/k groups
                        for a, b in zip(
                            vts + [None] * (4 - len(vts)), kfs, strict=True
                        ):
                            if a is not None:
                                work.append(a)
                            work.append(b)
                for fc in range(4):
                    posts = attention_pair(fc, j, qT_j, posts)
                    for wk in work[fc::4]:
                        wk()
            for p in posts:
                finalize_post(*p)
    nc.finalize()
    return nc


_NC_CACHE = {}


def _get_nc(key):
    if key not in _NC_CACHE:
        _NC_CACHE[key] = build_nc(*key)
    return _NC_CACHE[key]


def _pack_structure(mask):
    """Derive the packed-key program structure from the [B, S] 0/1 mask."""
    pos = [np.nonzero(mask[b])[0] for b in range(B)]
    nv = [len(p) for p in pos]
    SP = 128 * max(1, max((n + 127) // 128 for n in nv))
    NTP = SP // 128
    NKT = []
    for j in range(NQC):
        lim = 512 * j + 511
        nkt = max(
            max(1, (int(np.searchsorted(p, lim, side="right")) + 127) // 128)
            for p in pos
        )
        NKT.append(min(nkt, NTP))
    masked_offs = []
    for j in range(NQC):
        for kt in range(NKT[j]):
            fully_valid = True
            off = 512
            for b in range(B):
                r_lo, r_hi = kt * 128, kt * 128 + 127
                if r_hi >= nv[b] or pos[b][r_hi] > 512 * j:
                    fully_valid = False
                if r_lo >= nv[b] or pos[b][r_lo] > 512 * j + 511:
                    off_b = 512  # fully dead for this core
                else:
                    off_b = max(0, int(pos[b][r_lo]) - 512 * j)
                off = min(off, off_b)
            if not fully_valid:
                # kt==0 must write the full 512 columns so the PSUM ctx bank
                # is initialized everywhere (queries before the first valid
                # key produce all-zero E rows -> ctx 0 -> output 0, matching
                # the reference's 0/(0+1e-8))
                masked_offs.append(((j, kt), 0 if kt == 0 else min(off, 511)))
    return SP, tuple(NKT), tuple(masked_offs), pos, nv


def _host_trimask(masked_offs, pos_b, nv_b, SP):
    NM = max(len(masked_offs), 1)
    trim = np.zeros((NM, 128, 512), np.float32)
    p_full = np.full(SP, 1 << 30, np.int64)
    p_full[:nv_b] = pos_b
    qq = np.arange(512)
    for i, ((j, kt), _off) in enumerate(masked_offs):
        p_rows = p_full[kt * 128 : (kt + 1) * 128]
        trim[i] = (qq[None, :] + 512 * j >= p_rows[:, None]).astype(np.float32)
    return trim.astype(BF)


def kernel(Q, K, V, mask, W_Q, W_K, W_V, b_Q, b_K, b_V, _run=None):
    Q, K, V = (np.asarray(a, np.float32) for a in (Q, K, V))
    W_Q, W_K, W_V = (np.asarray(a, np.float32) for a in (W_Q, W_K, W_V))
    b_Q, b_K, b_V = (np.asarray(a, np.float32) for a in (b_Q, b_K, b_V))
    mask = np.asarray(mask)

    flags = (bool(b_Q.any()), bool(b_K.any()), bool(b_V.any()))
    SP, NKT, masked_offs, pos, nv = _pack_structure(mask)
    nc = _get_nc((flags, SP, NKT, masked_offs))

    in_maps = []
    for c in range(8):
        b, half = c // 2, c % 2
        fsl = slice(half * FPC, (half + 1) * FPC)
        Kp = np.zeros((SP, D), np.float32)
        Kp[: nv[b]] = K[b][pos[b]]
        Vp = np.zeros((SP, D), np.float32)
        Vp[: nv[b]] = V[b][pos[b]]
        m = {
            "xtq": np.ascontiguousarray(Q[b].T).astype(BF),
            "xtk": np.ascontiguousarray(Kp.T).astype(BF),
            "xtv": np.ascontiguousarray(Vp.T).astype(BF),
            "wtq": np.ascontiguousarray(W_Q[fsl].T).astype(BF),
            "wtk": np.ascontiguousarray(W_K[fsl].T).astype(BF),
            "wtv": np.ascontiguousarray(W_V[fsl].T).astype(BF),
            "trim": _host_trimask(masked_offs, pos[b], nv[b], SP),
            "ones": np.ones(128, np.float32),
            "onesb": np.ones(128, BF),
            "zeros": np.zeros(512, np.float32),
        }
        if flags[0]:
            m["bq"] = b_Q[fsl]
        if flags[1]:
            m["bk"] = b_K[fsl]
        if flags[2]:
            m["bv"] = b_V[:DV]
        in_maps.append(m)

    # the kernel broadcasts one [DV] b_V vector across heads; exact only when
    # b_V is constant across heads (it is zeros in this problem)
    if flags[2]:
        bv_heads = b_V.reshape(H, DV)
        assert np.allclose(bv_heads, bv_heads[0]), "per-head b_V unsupported"

    run = _run or (lambda n, im: run_bass_kernel_spmd(n, im, core_ids=list(range(8))))
    res = run(nc, in_maps)

    out = np.empty((B, S, H * DV), np.float32)
    for c in range(8):
        b, half = c // 2, c % 2
        out[b, :, half * FPC : (half + 1) * FPC] = res.results[c]["out"].T
    return out


# revision 45
# speedup vs baseline: 1.2123x; 1.2123x over previous
"""Trainium2 Bass kernel for nn_MultiHeadAttention (B=4, S=2048, D=1024,
H=16, DK=DV=64) with key-padding + causal mask, exp-without-max softmax.

Sharding: 8 cores = (batch b = core//2) x (head half = core%2, 8 heads each).
Each core computes its batch's projections for its 8 heads and the full
attention for those heads; host reassembles [B, S, H*DV].

Design (per core), all matmul operands bf16 (PSUM accumulates fp32):
 - KEY PACKING: the key-padding mask multiplies scores AFTER exp, so masked
   keys contribute nothing anywhere. The host gathers each batch's valid
   keys into a contiguous prefix (zero-padded to a fixed SP), so K/V
   projections and the whole attention run on ~51% of the keys. Causality
   against ORIGINAL key positions becomes per-batch 0/1 boundary tiles
   (host-precomputed) multiplied into E only where a tile straddles the
   causal boundary; fully-causal tiles need no mask at all. The k-tile trip
   counts / masked set / column trims are derived from the mask and baked
   into the program (cache-keyed).
 - host passes X^T = Q[b].T etc. [D, S] so the contraction dim (D) lands on
   partitions; W^T likewise. qT/kT computed as W^T-chunk.T @ X^T-chunk ->
   [feat, S]; v computed natural [S, feat].
 - mv1 = [v | 1] per k-tile: the ones column rides the PV matmul (M=65) and
   yields the masked denominator row for free (padded/masked rows have
   E == 0, so plain ones are exact).
 - scores^T tile [128 k, <=512 q] = kT-tile.T @ qT-chunk; ACT exp with
   scale=1/8 fused; boundary masks on DVE.
 - q^T per head zero-padded to K=128 partitions (the co-packed other
   head's kT rows are cancelled by the zeros; keeps the PE at full rate).
 - normalization without PE transposes: ctx^T stays feature-major. The
   sums row is DMA-regathered to [4,128], reciprocal_approx_fast'd,
   scattered into row 0 of a zeroed [128,512] tile, and broadcast to all
   partitions with one K=128 all-ones fp32r matmul; a DVE multiply then
   scales ctx^T and the output DMAs out feature-major. The host transposes
   the per-core [FPC, S] output during unshard.
 - software pipeline: project 512 tokens, then attend at that q-chunk for
   all heads, with the next group's projections emitted between head pairs
   so the Tile scheduler fills ACT-paced gaps with projection matmuls.
"""

import sys

sys.path.insert(0, "/opt/trn_rl_repo")

import ml_dtypes
import numpy as np

import concourse.bass as bass
import concourse.mybir as mybir
import concourse.tile as tile
from concourse import bacc
from concourse.bass_utils import run_bass_kernel_spmd

F32 = mybir.dt.float32
F32R = mybir.dt.float32r
BF16 = mybir.dt.bfloat16
EXP = mybir.ActivationFunctionType.Exp
COPY = mybir.ActivationFunctionType.Copy
IDENT_FN = mybir.ActivationFunctionType.Identity

B, S, D = 4, 2048, 1024
H, DK, DV = 16, 64, 64
HPC = 8  # heads per core
FPC = HPC * DK  # projected features per core (512)
NQC = S // 512  # 4 q-chunks
TC = 512  # projection token-chunk size
NDC = D // 128  # 8 contraction chunks
SCALE = 1.0 / np.sqrt(DK)
BF = np.dtype(ml_dtypes.bfloat16)


def build_nc(flags, SP, NKT, masked_offs):
    """flags: (has_bq, has_bk, has_bv). SP: padded packed-key count.
    NKT[j]: k-tile trip count for q-chunk j. masked_offs: ordered tuple of
    ((j, kt), off) for tiles needing a boundary mask, `off` = first valid
    q column (matmul/exp trimmed to [off:512])."""
    has_bq, has_bk, has_bv = flags
    NTP = SP // 128  # packed k tiles
    midx = {jk: i for i, (jk, _) in enumerate(masked_offs)}
    offs = dict(masked_offs)
    NM = len(masked_offs)
    # kv projection chunk widths
    KV_CW = [min(TC, SP - c * TC) for c in range((SP + TC - 1) // TC)]

    nc = bacc.Bacc()

    xt = {"q": nc.dram_tensor("xtq", [D, S], BF16, kind="ExternalInput")}
    for n in "kv":
        xt[n] = nc.dram_tensor(f"xt{n}", [D, SP], BF16, kind="ExternalInput")
    wt = {n: nc.dram_tensor(f"wt{n}", [D, FPC], BF16, kind="ExternalInput") for n in "qkv"}
    trim_d = nc.dram_tensor("trim", [max(NM, 1), 128, 512], BF16, kind="ExternalInput")
    ones_d = nc.dram_tensor("ones", [128], F32R, kind="ExternalInput")
    onesb_d = nc.dram_tensor("onesb", [128], BF16, kind="ExternalInput")
    zeros_d = nc.dram_tensor("zeros", [512], F32R, kind="ExternalInput")
    bq_d = nc.dram_tensor("bq", [FPC], F32, kind="ExternalInput") if has_bq else None
    bk_d = nc.dram_tensor("bk", [FPC], F32, kind="ExternalInput") if has_bk else None
    bv_d = nc.dram_tensor("bv", [DV], F32, kind="ExternalInput") if has_bv else None
    out_d = nc.dram_tensor("out", [FPC, S], F32R, kind="ExternalOutput")

    with tile.TileContext(nc) as tc:
        with (
            tc.tile_pool(name="const", bufs=1) as cpool,
            tc.tile_pool(name="wtp", bufs=1) as wtpool,
            tc.tile_pool(name="xtp", bufs=6) as xtpool,
            tc.tile_pool(name="big", bufs=1) as big,
            tc.tile_pool(name="qt", bufs=2) as qtpool,
            tc.tile_pool(name="e", bufs=4) as epool,
            tc.tile_pool(name="fin", bufs=4) as fin,
            tc.tile_pool(name="mm", bufs=6, space="PSUM") as psmm,
            tc.tile_pool(name="ctx", bufs=2, space="PSUM") as psctx,
        ):
            # ---------------- constants
            trim = cpool.tile([128, max(NM, 1), 512], BF16)
            # full 128-col all-ones weight (128 cols -> FWL-eligible LDW)
            ones128 = cpool.tile([128, 128], F32R)
            nc.sync.dma_start(
                out=ones128,
                in_=bass.AP(tensor=ones_d, offset=0, ap=[[0, 128], [1, 128]]),
            )
            ones_col = cpool.tile([128, 1], BF16)
            nc.sync.dma_start(
                out=ones_col,
                in_=bass.AP(tensor=onesb_d, offset=0, ap=[[1, 128], [128, 1]]),
            )
            # two alternating recip-scatter tiles (one per head of a pair) so
            # the deferred broadcast of pair p doesn't serialize pair p+1's
            # recip chain
            rec1z = []
            for i in range(2):
                t = cpool.tile([128, 512], F32R, name=f"rec1z{i}")
                nc.sync.dma_start(
                    out=t,
                    in_=bass.AP(tensor=zeros_d, offset=0, ap=[[0, 128], [1, 512]]),
                )
                rec1z.append(t)
            zeros_sb = cpool.tile([128, 512], F32)
            nc.vector.memset(zeros_sb, 0.0)
            bias_sb = {}
            for n, b_d in (("q", bq_d), ("k", bk_d)):
                if b_d is not None:
                    t = cpool.tile([128, 4], F32)
                    nc.sync.dma_start(
                        out=t, in_=bass.AP(tensor=b_d, offset=0, ap=[[1, 128], [128, 4]])
                    )
                    bias_sb[n] = t
            if bv_d is not None:
                bv_b = cpool.tile([128, FPC], F32)
                nc.sync.dma_start(
                    out=bv_b,
                    in_=bass.AP(tensor=bv_d, offset=0, ap=[[0, 128], [0, HPC], [1, DV]]),
                )

            # PE warm-up: dummy matmuls on the early-arriving (tiny) constant
            # tiles hold the PE activity monitor at full clock through the
            # initial input-DMA wait. Results never read; not DCE'd.
            for wi in range(24):
                wps = psmm.tile([128, 512], F32, tag="mm", name=f"warm{wi}")
                nc.tensor.matmul(
                    wps, lhsT=rec1z[0][:, 0:128], rhs=rec1z[0], start=True, stop=True
                )

            # persistent projection outputs (kT/mv1 span all packed tokens;
            # qT only lives for its own 512-token q-chunk group).
            # mv1 holds per-head [v | 1] at 65-col stride plus a 63-col zero
            # tail: the PV lhsT reads a 128-col window (spilling into the
            # next head's v; the resulting ctx rows 65..127 are never read)
            # so LDWEIGHTS is FWL-eligible and hides under the matmul stream.
            kT_all = big.tile([128, 4, SP], BF16)
            mv1 = big.tile([128, NTP, HPC * (DV + 1) + 63], BF16)
            for t in range(NTP):
                nc.vector.tensor_copy(
                    mv1[:, t, HPC * (DV + 1) :], zeros_sb[:, 0:63]
                )

            # weights loaded lazily at first use (interleaved per-dc with the
            # first x chunk so the first matmul's operands arrive first)
            w_sb = {}

            def dma_chunk(name, tcn, cw, sdim):
                """Allocate x tiles for one token chunk and issue its DMAs
                (plus the weight DMAs, dc-interleaved, on first use)."""
                first_w = name not in w_sb
                if first_w:
                    w_sb[name] = wtpool.tile(
                        [128, NDC, FPC], BF16, tag=f"w{name}", name=f"w{name}"
                    )
                halves = [
                    xtpool.tile([128, 4, TC], BF16, tag="x", name=f"x{name}{tcn}l"),
                    xtpool.tile([128, 4, TC], BF16, tag="x", name=f"x{name}{tcn}h"),
                ]
                x_at = lambda dc: halves[dc // 4][:, dc % 4, :cw]
                if first_w:
                    for dc in range(NDC):
                        nc.sync.dma_start(
                            out=w_sb[name][:, dc, :],
                            in_=bass.AP(
                                tensor=wt[name],
                                offset=dc * 128 * FPC,
                                ap=[[FPC, 128], [1, FPC]],
                            ),
                        )
                        nc.sync.dma_start(
                            out=x_at(dc),
                            in_=bass.AP(
                                tensor=xt[name],
                                offset=tcn * TC + dc * 128 * sdim,
                                ap=[[sdim, 128], [1, cw]],
                            ),
                        )
                else:
                    for hv in range(2):
                        nc.sync.dma_start(
                            out=halves[hv][:, :, :cw],
                            in_=bass.AP(
                                tensor=xt[name],
                                offset=tcn * TC + hv * 4 * 128 * sdim,
                                ap=[[sdim, 128], [128 * sdim, 4], [1, cw]],
                            ),
                        )
                return x_at

            def q_fc_group(x_at, fc, qT_j):
                ps = psmm.tile([128, TC], F32, tag="mm")
                for dc in range(NDC):
                    nc.tensor.matmul(
                        ps,
                        lhsT=w_sb["q"][:, dc, fc * 128 : (fc + 1) * 128],
                        rhs=x_at(dc),
                        start=(dc == 0),
                        stop=(dc == NDC - 1),
                    )
                # split into per-head zero-padded blocks (DVE keeps ACT
                # free for exp)
                for hh in range(2):
                    hp = hh * 64
                    o = qT_j[hp : hp + 64, 2 * fc + hh, :]
                    i_ = ps[hp : hp + 64, :]
                    if "q" in bias_sb:
                        nc.vector.tensor_scalar_add(
                            o, i_, bias_sb["q"][hp : hp + 64, fc : fc + 1]
                        )
                    else:
                        nc.vector.tensor_copy(o, i_)

            def k_fc_group(x_at, tcn, cw, fc):
                ps = psmm.tile([128, TC], F32, tag="mm")
                for dc in range(NDC):
                    nc.tensor.matmul(
                        ps[:, :cw],
                        lhsT=w_sb["k"][:, dc, fc * 128 : (fc + 1) * 128],
                        rhs=x_at(dc),
                        start=(dc == 0),
                        stop=(dc == NDC - 1),
                    )
                o = kT_all[:, fc, tcn * TC : tcn * TC + cw]
                if "k" in bias_sb:
                    nc.scalar.activation(
                        o, ps[:, :cw], IDENT_FN, bias=bias_sb["k"][:, fc : fc + 1]
                    )
                else:
                    nc.scalar.activation(o, ps[:, :cw], COPY)

            def v_tt_group(x_at, tcn, tt):
                t = tcn * (TC // 128) + tt
                ps = psmm.tile([128, FPC], F32, tag="mm")
                for dc in range(NDC):
                    nc.tensor.matmul(
                        ps,
                        lhsT=x_at(dc)[:, tt * 128 : (tt + 1) * 128],
                        rhs=w_sb["v"][:, dc, :],
                        start=(dc == 0),
                        stop=(dc == NDC - 1),
                    )
                if bv_d is not None:
                    nc.vector.tensor_add(ps, ps, bv_b)
                for h in range(HPC):
                    base = h * (DV + 1)
                    nc.vector.tensor_copy(
                        mv1[:, t, base : base + DV], ps[:, h * DV : (h + 1) * DV]
                    )
                    nc.gpsimd.tensor_copy(
                        mv1[:, t, base + DV : base + DV + 1], ones_col
                    )

            def project_q_chunk(tcn, qT_j):
                x_at = dma_chunk("q", tcn, TC, S)
                for fc in range(4):
                    q_fc_group(x_at, fc, qT_j)

            def project_kv_chunk(tcn):
                cw = KV_CW[tcn]
                x_at_v = dma_chunk("v", tcn, cw, SP)
                for tt in range(cw // 128):
                    v_tt_group(x_at_v, tcn, tt)
                x_at_k = dma_chunk("k", tcn, cw, SP)
                for fc in range(4):
                    k_fc_group(x_at_k, tcn, cw, fc)

            def finalize_pre(h, j, hh, ctx):
                """Move ctx to SBUF (releases the PSUM bank) and start the
                off-PE recip chain: sums regather -> +eps -> approx recip ->
                scatter into rec1z[hh] row 0."""
                ctx_sb = fin.tile([DV + 1, 512], F32R, tag="ctxsb", bufs=4)
                nc.vector.tensor_copy(ctx_sb, ctx[0 : DV + 1, :])
                sums4 = fin.tile([4, 128], F32R, tag="sums", bufs=2)
                nc.sync.dma_start(
                    out=sums4,
                    in_=ctx_sb[DV : DV + 1, :].rearrange("p (a b) -> p a b", a=4),
                )
                rec = fin.tile([4, 128], F32, tag="rec", bufs=2)
                nc.vector.tensor_scalar_add(rec, sums4, 1e-8)
                rec2 = fin.tile([4, 128], F32, tag="rec2", bufs=2)
                nc.vector.reciprocal_approx_fast(out=rec2, in_=rec)
                # gpsimd DGE: the f32 -> f32r retag counts as a casting DMA
                nc.gpsimd.dma_start(
                    out=rec1z[hh][0:1, :].rearrange("p (a b) -> p a b", a=4),
                    in_=rec2,
                )
                return (h, j, hh, ctx_sb)

            def finalize_post(h, j, hh, ctx_sb):
                """Broadcast recip to all partitions with an all-ones matmul,
                scale ctx^T, and DMA the feature-major block out. Emitted one
                attention-pair late so the recip chain's DMA latency hides
                behind the next pair's matmuls instead of stalling the PE."""
                bps = psmm.tile([128, 512], F32, tag="mm", name=f"bc{h}_{j}")
                nc.tensor.matmul(
                    bps, lhsT=ones128, rhs=rec1z[hh], start=True, stop=True
                )
                out_sb = fin.tile([DV, 512], F32R, tag="outsb", bufs=6)
                nc.vector.tensor_mul(out_sb, ctx_sb[0:DV, :], bps[0:DV, :])
                nc.sync.dma_start(
                    out=out_d[h * DV : (h + 1) * DV, j * 512 : (j + 1) * 512],
                    in_=out_sb,
                )

            def attention_pair(fc, j, qT_j, posts):
                h0, h1 = 2 * fc, 2 * fc + 1
                nkt = NKT[j]
                ctxs = [
                    psctx.tile([128, 512], F32, tag="ctx", name=f"ctx{fc}_{j}_{hh}")
                    for hh in range(2)
                ]
                def emit_pv(kt, off, Es):
                    for hh, h in enumerate((h0, h1)):
                        base = h * (DV + 1)
                        nc.tensor.matmul(
                            ctxs[hh][:, off:],
                            lhsT=mv1[:, kt, base : base + 128],
                            rhs=Es[hh][:, off:],
                            start=(kt == 0),
                            stop=(kt == nkt - 1),
                        )

                # one-step QK lookahead: kt+1's QK is emitted before kt's PV
                # so the PE always has a ready matmul while exp/mask run
                pend = None
                for kt in range(nkt):
                    mi = midx.get((j, kt))
                    off = offs.get((j, kt), 0)
                    qs = slice(off, 512)
                    scs = []
                    for hh, h in enumerate((h0, h1)):
                        sc = psmm.tile([128, 512], F32, tag="mm", name=f"sc{fc}_{j}_{kt}_{hh}")
                        nc.tensor.matmul(
                            sc[:, off:],
                            lhsT=kT_all[:, fc, kt * 128 : (kt + 1) * 128],
                            rhs=qT_j[:, h, qs],
                            start=True,
                            stop=True,
                        )
                        scs.append(sc)
                    Es = []
                    for hh, h in enumerate((h0, h1)):
                        E = epool.tile([128, 512], BF16, tag="e", name=f"e{fc}_{j}_{kt}_{hh}")
                        nc.scalar.activation(E[:, off:], scs[hh][:, off:], EXP, scale=float(SCALE))
                        if mi is not None:
                            nc.vector.tensor_mul(E[:, off:], E[:, off:], trim[:, mi, off:])
                        Es.append(E)
                    if pend is not None:
                        emit_pv(*pend)
                    pend = (kt, off, Es)
                emit_pv(*pend)
                # previous pair's deferred finalize lands behind this pair's
                # matmuls in the PE queue
                for p in posts:
                    finalize_post(*p)
                return [
                    finalize_pre(h, j, hh, ctxs[hh]) for hh, h in enumerate((h0, h1))
                ]

            # software pipeline: project 512 tokens, then attend at that
            # q-chunk for all heads (next group's projections fill PE gaps).
            # q^T stored per head zero-padded to K=128 partitions: a full-K
            # matmul keeps the PE clock-gate warm and the co-packed other
            # head's kT rows are cancelled by the zeros.
            def new_qtile(j):
                qT_j = qtpool.tile([128, HPC, 512], BF16, tag="qt", name=f"qt{j}")
                for h in range(HPC):
                    zp = 64 - (h % 2) * 64
                    nc.vector.tensor_copy(qT_j[zp : zp + 64, h, :], zeros_sb[0:64, :])
                return qT_j

            NKC = len(KV_CW)
            # kv chunk c is first needed by the q-chunk j where NKT[j] > 4c
            kv_need = {}
            for c in range(NKC):
                js = [j for j in range(NQC) if NKT[j] > 4 * c]
                kv_need[c] = js[0] if js else None

            qtiles = {0: new_qtile(0)}
            project_q_chunk(0, qtiles[0])
            for c in range(NKC):
                if kv_need[c] == 0 or kv_need[c] is None:
                    project_kv_chunk(c)
            # boundary-mask tiles aren't needed until the first attention
            # pair; issuing their (large) DMA after the upfront projection
            # DMAs keeps the pipeline head tight
            nc.sync.dma_start(
                out=trim,
                in_=bass.AP(
                    tensor=trim_d,
                    offset=0,
                    ap=[[512, 128], [512 * 128, max(NM, 1)], [1, 512]],
                ),
            )
            posts = []
            for j in range(NQC):
                qT_j = qtiles[j]
                # stage next-chunk projections as a work list spread evenly
                # across this j's four attention pairs (PE filler during the
                # ACT-paced attention stretches)
                work = []
                if j + 1 < NQC:
                    qtiles[j + 1] = new_qtile(j + 1)
                    x_at_q = dma_chunk("q", j + 1, TC, S)
                    work += [
                        (lambda fc=fc: q_fc_group(x_at_q, fc, qtiles[j + 1]))
                        for fc in range(4)
                    ]
                for c in range(NKC):
                    if kv_need[c] == j + 1:
                        cw = KV_CW[c]
                        x_at_v = dma_chunk("v", c, cw, SP)
                        x_at_k = dma_chunk("k", c, cw, SP)
                        vts = [
                            (lambda tt=tt, x=x_at_v, c=c: v_tt_group(x, c, tt))
                            for tt in range(cw // 128)
                        ]
                        kfs = [
                            (lambda fc=fc, x=x_at_k, c=c, cw=cw: k_fc_group(x, c, cw, fc))
                            for fc in range(4)
                        ]
                        # interleave v/k groups
                        for a, b in zip(
                            vts + [None] * (4 - len(vts)), kfs, strict=True
                        ):
                            if a is not None:
                                work.append(a)
                            work.append(b)
                for fc in range(4):
                    posts = attention_pair(fc, j, qT_j, posts)
                    for wk in work[fc::4]:
                        wk()
            for p in posts:
                finalize_post(*p)
    nc.finalize()
    return nc


_NC_CACHE = {}


def _get_nc(key):
    if key not in _NC_CACHE:
        _NC_CACHE[key] = build_nc(*key)
    return _NC_CACHE[key]


def _pack_structure(mask):
    """Derive the packed-key program structure from the [B, S] 0/1 mask."""
    pos = [np.nonzero(mask[b])[0] for b in range(B)]
    nv = [len(p) for p in pos]
    SP = 128 * max(1, max((n + 127) // 128 for n in nv))
    NTP = SP // 128
    NKT = []
    for j in range(NQC):
        lim = 512 * j + 511
        nkt = max(
            max(1, (int(np.searchsorted(p, lim, side="right")) + 127) // 128)
            for p in pos
        )
        NKT.append(min(nkt, NTP))
    masked_offs = []
    for j in range(NQC):
        for kt in range(NKT[j]):
            fully_valid = True
            off = 512
            for b in range(B):
                r_lo, r_hi = kt * 128, kt * 128 + 127
                if r_hi >= nv[b] or pos[b][r_hi] > 512 * j:
                    fully_valid = False
                if r_lo >= nv[b] or pos[b][r_lo] > 512 * j + 511:
                    off_b = 512  # fully dead for this core
                else:
                    off_b = max(0, int(pos[b][r_lo]) - 512 * j)
                off = min(off, off_b)
            if not fully_valid:
                # kt==0 must write the full 512 columns so the PSUM ctx bank
                # is initialized everywhere (queries before the first valid
                # key produce all-zero E rows -> ctx 0 -> output 0, matching
                # the reference's 0/(0+1e-8))
                masked_offs.append(((j, kt), 0 if kt == 0 else min(off, 511)))
    return SP, tuple(NKT), tuple(masked_offs), pos, nv


def _host_trimask(masked_offs, pos_b, nv_b, SP):
    NM = max(len(masked_offs), 1)
    trim = np.zeros((NM, 128, 512), np.float32)
    p_full = np.full(SP, 1 << 30, np.int64)
    p_full[:nv_b] = pos_b
    qq = np.arange(512)
    for i, ((j, kt), _off) in enumerate(masked_offs):
        p_rows = p_full[kt * 128 : (kt + 1) * 128]
        trim[i] = (qq[None, :] + 512 * j >= p_rows[:, None]).astype(np.float32)
    return trim.astype(BF)


def kernel(Q, K, V, mask, W_Q, W_K, W_V, b_Q, b_K, b_V, _run=None):
    Q, K, V = (np.asarray(a, np.float32) for a in (Q, K, V))
    W_Q, W_K, W_V = (np.asarray(a, np.float32) for a in (W_Q, W_K, W_V))
    b_Q, b_K, b_V = (np.asarray(a, np.float32) for a in (b_Q, b_K, b_V))
    mask = np.asarray(mask)

    flags = (bool(b_Q.any()), bool(b_K.any()), bool(b_V.any()))
    SP, NKT, masked_offs, pos, nv = _pack_structure(mask)
    nc = _get_nc((flags, SP, NKT, masked_offs))

    in_maps = []
    for c in range(8):
        b, half = c // 2, c % 2
        fsl = slice(half * FPC, (half + 1) * FPC)
        Kp = np.zeros((SP, D), np.float32)
        Kp[: nv[b]] = K[b][pos[b]]
        Vp = np.zeros((SP, D), np.float32)
        Vp[: nv[b]] = V[b][pos[b]]
        m = {
            "xtq": np.ascontiguousarray(Q[b].T).astype(BF),
            "xtk": np.ascontiguousarray(Kp.T).astype(BF),
            "xtv": np.ascontiguousarray(Vp.T).astype(BF),
            "wtq": np.ascontiguousarray(W_Q[fsl].T).astype(BF),
            "wtk": np.ascontiguousarray(W_K[fsl].T).astype(BF),
            "wtv": np.ascontiguousarray(W_V[fsl].T).astype(BF),
            "trim": _host_trimask(masked_offs, pos[b], nv[b], SP),
            "ones": np.ones(128, np.float32),
            "onesb": np.ones(128, BF),
            "zeros": np.zeros(512, np.float32),
        }
        if flags[0]:
            m["bq"] = b_Q[fsl]
        if flags[1]:
            m["bk"] = b_K[fsl]
        if flags[2]:
            m["bv"] = b_V[:DV]
        in_maps.append(m)

    # the kernel broadcasts one [DV] b_V vector across heads; exact only when
    # b_V is constant across heads (it is zeros in this problem)
    if flags[2]:
        bv_heads = b_V.reshape(H, DV)
        assert np.allclose(bv_heads, bv_heads[0]), "per-head b_V unsupported"

    run = _run or (lambda n, im: run_bass_kernel_spmd(n, im, core_ids=list(range(8))))
    res = run(nc, in_maps)

    out = np.empty((B, S, H * DV), np.float32)
    for c in range(8):
        b, half = c // 2, c % 2
        out[b, :, half * FPC : (half + 1) * FPC] = res.results[c]["out"].T
    return out
